# revision 2
# baseline (speedup 1.0000x reference)
"""BitNetLinear (ternary eval-mode) forward on 8 trn2 NeuronCores.

Math (reference):
    s_w  = max(mean|W|, eps);  q = sign(W) * (|W/s_w| > 0.5)
    s_x  = max(mean|x|, eps)
    out  = (x/s_x) @ (q*s_w)^T * s_x + bias * s_x
         = x @ q^T * s_w + bias * s_x          (exact in real arithmetic)

Sharding: 2D grid, TG=4 token groups x FG=2 out-feature groups.
Each core: T=1024 tokens, O=2048 out features, I=4096 contraction.
s_w needs a global view of W -> each core reduces |.| over a distinct
1/8 of W and a 1-scalar AllReduce produces the global sum. bias*s_x is
added on the host (bias is identically zero for this problem, so the
term contributes nothing; host uses the exact reference formula).

Kernel pipeline per core:
  - sum pass over its W-eighth (DVE abs-reduce, PE ones-matmul to scalar)
  - AllReduce(add) of the scalar; s_w/thresholds derived on-chip
  - x cast to bf16 + PE-transpose into resident x^T tiles
  - per 512-wide o-chunk: quantize W strips to ternary bf16
    (-q = (w < -thr) - (w > thr), POOL+DVE), PE-transpose to q^T,
    then matmul sweep (stationary = x^T tile, moving = q^T, fp32 PSUM),
    eviction scales by -s_w on ACT.
"""

import sys

sys.path.insert(0, "/opt/trn_rl_repo")

import numpy as np

P = 128
EPS = 1e-8

B, S = 2, 2048
I_FULL = 4096  # in_features
O_FULL = 4096  # out_features
N_CORES = 8
TG, FG = 4, 2
T_SH = (B * S) // TG  # 1024
O_SH = O_FULL // FG  # 2048
W8_ROWS = O_FULL // N_CORES  # 512


def build_nc(T, O, I, n_cores, w8_rows, w_elems_total):
    """Build + compile the SPMD Bass module for one core shape."""
    from concourse import bacc, mybir, tile
    import concourse.bass as bass
    from concourse.bass import ts, ds
    from concourse.masks import make_identity

    f32 = mybir.dt.float32
    bf16 = mybir.dt.bfloat16
    A = mybir.AluOpType

    assert T % P == 0 and O % P == 0 and I % (4 * P) == 0

    nc = bacc.Bacc(
        "TRN2", target_bir_lowering=False, debug=False, num_devices=n_cores
    )
    x_sh = nc.dram_tensor("x_sh", [T, I], f32, kind="ExternalInput").ap()
    w_half = nc.dram_tensor("w_half", [O, I], f32, kind="ExternalInput").ap()
    w8 = nc.dram_tensor("w8", [w8_rows, I], f32, kind="ExternalInput").ap()
    out_sh = nc.dram_tensor("out_sh", [T, O], f32, kind="ExternalOutput").ap()

    n_tb = T // P
    n_ib = I // P
    OC = min(512, O)  # o-chunk width
    n_oc = O // OC
    n_os = OC // P  # o strips per chunk (4)
    IH = I // 2  # i-half width for quant strips
    XQ = I // 4  # x quarter width
    n_w8r = w8_rows // P

    with tile.TileContext(nc) as tc:
        with (
            tc.tile_pool(name="const", bufs=1) as const_pool,
            tc.tile_pool(name="scal", bufs=1) as scal_pool,
            tc.tile_pool(name="dram", bufs=1, space="DRAM") as dram_pool,
            tc.tile_pool(name="xin", bufs=3) as xin_pool,
            tc.tile_pool(name="xbq", bufs=2) as xb_pool,
            tc.tile_pool(name="xt", bufs=1) as xt_pool,
            tc.tile_pool(name="win", bufs=2) as win_pool,
            tc.tile_pool(name="tq", bufs=2) as tq_pool,
            tc.tile_pool(name="qn", bufs=2) as qn_pool,
            tc.tile_pool(name="qt", bufs=2) as qt_pool,
            tc.tile_pool(name="osb", bufs=3) as out_pool,
            tc.tile_pool(name="psacc", bufs=2, space="PSUM") as ps_acc,
            tc.tile_pool(name="pstr", bufs=2, space="PSUM") as ps_tr,
            tc.tile_pool(name="pssum", bufs=1, space="PSUM") as ps_sum,
        ):
            identity = const_pool.tile([P, P], bf16)
            make_identity(nc, identity[:])
            ones = const_pool.tile([P, 1], f32)
            nc.vector.memset(ones[:], 1.0)

            # ---- phase S: partial sum of |W| over this core's eighth ----
            n_sum_strips = n_w8r * 2
            acc = scal_pool.tile([P, n_sum_strips], f32)
            for r in range(n_w8r):
                for h in range(2):
                    wst = win_pool.tile([P, IH], f32, tag="w")
                    nc.sync.dma_start(wst[:], w8[ts(r, P), ds(h * IH, IH)])
                    nc.vector.tensor_reduce(
                        acc[:, r * 2 + h : r * 2 + h + 1],
                        wst[:],
                        axis=mybir.AxisListType.X,
                        op=A.add,
                        apply_absolute_value=True,
                    )
            red = scal_pool.tile([P, 1], f32)
            nc.vector.tensor_reduce(
                red[:], acc[:], axis=mybir.AxisListType.X, op=A.add
            )
            ps_s = ps_sum.tile([1, 1], f32)
            nc.tensor.matmul(ps_s[:], lhsT=red[:], rhs=ones[:], start=True, stop=True)
            sb_s = scal_pool.tile([1, 1], f32)
            nc.scalar.copy(sb_s[:], ps_s[:])

            # ---- phase C: AllReduce the scalar across all cores ----
            cc_in = dram_pool.tile([1, 1], f32)
            cc_out = dram_pool.tile([1, 1], f32)
            nc.sync.dma_start(cc_in[:], sb_s[:])
            nc.gpsimd.collective_compute(
                "AllReduce",
                A.add,
                replica_groups=[list(range(n_cores))],
                ins=[cc_in[:]],
                outs=[cc_out[:]],
            )
            cc_out_ap = cc_out[:]
            bcast_ap = bass.AP(
                tensor=cc_out_ap.tensor,
                offset=cc_out_ap.offset,
                ap=[[0, P], [1, 1]],
            )
            s_sum = scal_pool.tile([P, 1], f32)
            nc.sync.dma_start(s_sum[:], bcast_ap)
            sw = scal_pool.tile([P, 1], f32)
            nc.vector.tensor_scalar(
                out=sw[:],
                in0=s_sum[:],
                scalar1=1.0 / float(w_elems_total),
                scalar2=EPS,
                op0=A.mult,
                op1=A.max,
            )
            thr = scal_pool.tile([P, 1], f32)
            nc.vector.tensor_scalar(
                out=thr[:], in0=sw[:], scalar1=0.5, scalar2=None, op0=A.mult
            )
            nthr = scal_pool.tile([P, 1], f32)
            nc.vector.tensor_scalar(
                out=nthr[:], in0=sw[:], scalar1=-0.5, scalar2=None, op0=A.mult
            )
            nsw = scal_pool.tile([P, 1], f32)
            nc.vector.tensor_scalar(
                out=nsw[:], in0=sw[:], scalar1=-1.0, scalar2=None, op0=A.mult
            )

            # ---- phase X: x -> bf16 -> x^T tiles (independent of C) ----
            xt_tiles = {}
            for tb in range(n_tb):
                for xq in range(I // XQ):
                    xqt = xin_pool.tile([P, XQ], f32, tag="x")
                    nc.sync.dma_start(xqt[:], x_sh[ts(tb, P), ds(xq * XQ, XQ)])
                    xbq = xb_pool.tile([P, XQ], bf16, tag="xb")
                    nc.scalar.copy(xbq[:], xqt[:])
                    for j in range(XQ // P):
                        ib = xq * (XQ // P) + j
                        pst = ps_tr.tile([P, P], bf16, tag="xt_ps")
                        nc.tensor.transpose(pst[:], xbq[:, ts(j, P)], identity[:])
                        xt = xt_pool.tile([P, P], bf16, tag=f"xt_{ib}_{tb}")
                        if (ib + tb) % 2 == 0:
                            nc.vector.tensor_copy(xt[:], pst[:])
                        else:
                            nc.scalar.copy(xt[:], pst[:])
                        xt_tiles[(ib, tb)] = xt

            # ---- per o-chunk: quantize W + transpose, then matmul sweep ----
            for c in range(n_oc):
                qt_tiles = []
                for ib in range(n_ib):
                    qt_tiles.append(
                        qt_pool.tile(
                            [P, OC], bf16, tag=f"qt_{ib}", name=f"qt_{c}_{ib}"
                        )
                    )
                for s in range(n_os):
                    r0 = c * OC + s * P
                    qnst = qn_pool.tile([P, I], bf16, tag="qn")
                    for h in range(2):
                        wst = win_pool.tile([P, IH], f32, tag="w")
                        nc.sync.dma_start(wst[:], w_half[ds(r0, P), ds(h * IH, IH)])
                        tst = tq_pool.tile([P, IH], bf16, tag="t")
                        nc.gpsimd.tensor_scalar(
                            out=tst[:],
                            in0=wst[:],
                            scalar1=thr[:],
                            scalar2=None,
                            op0=A.is_gt,
                        )
                        # -q = (w < -thr) - (w > thr)  in {-1,0,1} bf16
                        nc.vector.scalar_tensor_tensor(
                            out=qnst[:, ds(h * IH, IH)],
                            in0=wst[:],
                            scalar=nthr[:],
                            in1=tst[:],
                            op0=A.is_lt,
                            op1=A.subtract,
                        )
                    for ib in range(n_ib):
                        psq = ps_tr.tile([P, P], bf16, tag="qt_ps")
                        nc.tensor.transpose(psq[:], qnst[:, ts(ib, P)], identity[:])
                        if (ib + s) % 2 == 0:
                            nc.scalar.copy(qt_tiles[ib][:, ts(s, P)], psq[:])
                        else:
                            nc.vector.tensor_copy(qt_tiles[ib][:, ts(s, P)], psq[:])
                for tb in range(n_tb):
                    ps = ps_acc.tile([P, OC], f32, tag="acc")
                    for ib in range(n_ib):
                        nc.tensor.matmul(
                            ps[:],
                            lhsT=xt_tiles[(ib, tb)][:],
                            rhs=qt_tiles[ib][:],
                            start=(ib == 0),
                            stop=(ib == n_ib - 1),
                        )
                    osb = out_pool.tile([P, OC], f32, tag="o")
                    # out = psum * (-s_w)   (psum holds x @ (-q)^T)
                    nc.scalar.activation(
                        osb[:],
                        ps[:],
                        mybir.ActivationFunctionType.Copy,
                        scale=nsw[:],
                    )
                    nc.sync.dma_start(out_sh[ts(tb, P), ds(c * OC, OC)], osb[:])

    nc.compile()
    return nc


_CACHE = {}


def _get_nc(key):
    if key not in _CACHE:
        _CACHE[key] = build_nc(*key)
    return _CACHE[key]


def run(x2d, weight, n_cores=N_CORES, tg=TG, fg=FG):
    """Run the sharded device computation: returns x @ q^T * s_w, [Ttot, O_full]."""
    from concourse.bass_utils import run_bass_kernel_spmd

    t_tot, i_full = x2d.shape
    o_full = weight.shape[0]
    t_sh = t_tot // tg
    o_sh = o_full // fg
    w8_rows = o_full // n_cores
    key = (t_sh, o_sh, i_full, n_cores, w8_rows, o_full * i_full)
    nc = _get_nc(key)

    in_maps = []
    for cid in range(n_cores):
        g, b = cid // fg, cid % fg
        in_maps.append(
            {
                "x_sh": np.ascontiguousarray(x2d[g * t_sh : (g + 1) * t_sh]),
                "w_half": np.ascontiguousarray(weight[b * o_sh : (b + 1) * o_sh]),
                "w8": np.ascontiguousarray(
                    weight[b * o_sh + g * w8_rows : b * o_sh + (g + 1) * w8_rows]
                ),
            }
        )
    res = run_bass_kernel_spmd(nc, in_maps, core_ids=list(range(n_cores)))
    out = np.empty((t_tot, o_full), np.float32)
    for cid in range(n_cores):
        g, b = cid // fg, cid % fg
        out[g * t_sh : (g + 1) * t_sh, b * o_sh : (b + 1) * o_sh] = res.results[
            cid
        ]["out_sh"]
    return out


def kernel(x, weight, bias):
    x = np.asarray(x, np.float32)
    weight = np.asarray(weight, np.float32)
    bias = np.asarray(bias, np.float32)
    t_tot = x.shape[0] * x.shape[1]
    out = run(x.reshape(t_tot, x.shape[2]), weight)
    # bias term: out += bias * s_x (exact reference semantics; zero for
    # this problem's bias). The matmul term is s_x-invariant.
    if np.any(bias):
        s_x = np.float32(max(np.mean(np.abs(x)), EPS))
        out = out + (bias * s_x)[None, :]
    return out.reshape(x.shape[0], x.shape[1], weight.shape[0])


# revision 6
# speedup vs baseline: 3.1062x; 3.1062x over previous
"""BitNetLinear (ternary eval-mode) forward on 8 trn2 NeuronCores.

Math (reference):
    s_w  = max(mean|W|, eps);  q = sign(W) * (|W/s_w| > 0.5)
    s_x  = max(mean|x|, eps)
    out  = (x/s_x) @ (q*s_w)^T * s_x + bias * s_x
         = x @ q^T * s_w + bias * s_x          (exact in real arithmetic)

Sharding: 2D grid, TG=4 token groups x FG=2 out-feature groups.
Each core: T=1024 tokens, O=2048 out features, I=4096 contraction.
Host passes x and W shards PRE-TRANSPOSED (i-major) so both matmul
operands already have the contraction dim on partitions — no on-chip
transposes. s_w needs a global view of W: each core reduces |.| over a
distinct 1/8 of W and a 1-scalar AllReduce(add) produces the global
sum. bias*s_x is added on the host (bias is identically zero for this
problem; host uses the exact reference formula).

Device pipeline per core:
  - |W| partial sum over its eighth (DVE abs-reduce + PE ones-matmul)
  - AllReduce scalar -> s_w, thr = 0.5*s_w on chip
  - x^T strips: DMA f32, ACT cast -> resident bf16 tiles [128i, T]
  - per 512-wide o-chunk, per i-block: DMA w^T strip [128i, 512o],
    quantize to 2q in {-2,0,2} bf16 via
        t2 = (w > thr) * 2          (DVE tensor_scalar, fused dual op)
        s2 = Sign(w + thr)          (ACT activation)
        q2 = (t2 - 1) + s2          (DVE scalar_tensor_tensor)
    then matmul sweep: psum[t,o] += xT.T @ q2T (fp32 PSUM, K=4096)
    and evict with scale thr (= s_w/2, undoing the 2x) on ACT.
"""

import sys

sys.path.insert(0, "/opt/trn_rl_repo")

import numpy as np

P = 128
EPS = 1e-8

B, S = 2, 2048
I_FULL = 4096  # in_features
O_FULL = 4096  # out_features
N_CORES = 8
TG, FG = 4, 2
T_SH = (B * S) // TG  # 1024
O_SH = O_FULL // FG  # 2048


def build_nc(T, O, I, n_cores, tg, w_elems_total):
    """Build + compile the SPMD Bass module for one core shape."""
    from concourse import bacc, mybir, tile
    import concourse.bass as bass
    from concourse.bass import ts, ds

    f32 = mybir.dt.float32
    bf16 = mybir.dt.bfloat16
    A = mybir.AluOpType

    assert T % P == 0 and O % P == 0 and I % P == 0

    nc = bacc.Bacc(
        "TRN2", target_bir_lowering=False, debug=False, num_devices=n_cores
    )
    # all inputs pre-transposed on host: i-major
    xT = nc.dram_tensor("xT", [I, T], f32, kind="ExternalInput").ap()
    wT = nc.dram_tensor("wT", [I, O], f32, kind="ExternalInput").ap()
    out_sh = nc.dram_tensor("out_sh", [T, O], f32, kind="ExternalOutput").ap()

    n_tb = T // P
    n_ib = I // P
    OC = min(512, O)  # o-chunk width
    n_oc = O // OC
    i_slab = I // tg  # rows of wT this core abs-sums

    with tile.TileContext(nc) as tc:
        with (
            tc.tile_pool(name="scal", bufs=1) as scal_pool,
            tc.tile_pool(name="dram", bufs=1, space="DRAM") as dram_pool,
            tc.tile_pool(name="sumw", bufs=2) as sum_pool,
            tc.tile_pool(name="xin", bufs=3) as xin_pool,
            tc.tile_pool(name="xt", bufs=1) as xt_pool,
            tc.tile_pool(name="win", bufs=3) as win_pool,
            tc.tile_pool(name="tq", bufs=3) as tq_pool,
            tc.tile_pool(name="sq", bufs=3) as sq_pool,
            tc.tile_pool(name="qt", bufs=2) as qt_pool,
            tc.tile_pool(name="osb", bufs=4) as out_pool,
            tc.tile_pool(name="psacc", bufs=4, space="PSUM") as ps_acc,
            tc.tile_pool(name="pssum", bufs=1, space="PSUM") as ps_sum,
        ):
            ones = scal_pool.tile([P, 1], f32)
            nc.vector.memset(ones[:], 1.0)

            # ---- phase S: partial sum of |W| over this core's i-slab ----
            # slab rows depend on the core; host passes the slab as the
            # FIRST i_slab rows? No — host rotates wT per core so that
            # rows [0, i_slab) are this core's slab (see run()).
            n_sum = i_slab // P
            acc = scal_pool.tile([P, n_sum], f32)
            for r in range(n_sum):
                wst = sum_pool.tile([P, O], f32, tag="ws")
                nc.sync.dma_start(wst[:], wT[ts(r, P), :])
                nc.vector.tensor_reduce(
                    acc[:, r : r + 1],
                    wst[:],
                    axis=mybir.AxisListType.X,
                    op=A.add,
                    apply_absolute_value=True,
                )
            red = scal_pool.tile([P, 1], f32)
            nc.vector.tensor_reduce(
                red[:], acc[:], axis=mybir.AxisListType.X, op=A.add
            )
            ps_s = ps_sum.tile([1, 1], f32)
            nc.tensor.matmul(ps_s[:], lhsT=red[:], rhs=ones[:], start=True, stop=True)
            sb_s = scal_pool.tile([1, 1], f32)
            nc.scalar.copy(sb_s[:], ps_s[:])

            # ---- phase C: AllReduce the scalar across all cores ----
            cc_in = dram_pool.tile([1, 1], f32)
            cc_out = dram_pool.tile([1, 1], f32)
            nc.sync.dma_start(cc_in[:], sb_s[:])
            nc.gpsimd.collective_compute(
                "AllReduce",
                A.add,
                replica_groups=[list(range(n_cores))],
                ins=[cc_in[:]],
                outs=[cc_out[:]],
            )
            cc_out_ap = cc_out[:]
            bcast_ap = bass.AP(
                tensor=cc_out_ap.tensor,
                offset=cc_out_ap.offset,
                ap=[[0, P], [1, 1]],
            )
            s_sum = scal_pool.tile([P, 1], f32)
            nc.sync.dma_start(s_sum[:], bcast_ap)
            sw = scal_pool.tile([P, 1], f32)
            nc.vector.tensor_scalar(
                out=sw[:],
                in0=s_sum[:],
                scalar1=1.0 / float(w_elems_total),
                scalar2=EPS,
                op0=A.mult,
                op1=A.max,
            )
            thr = scal_pool.tile([P, 1], f32)
            nc.vector.tensor_scalar(
                out=thr[:], in0=sw[:], scalar1=0.5, scalar2=None, op0=A.mult
            )

            # ---- phase X: x^T strips -> resident bf16 tiles ----
            xt_tiles = []
            for ib in range(n_ib):
                xq = xin_pool.tile([P, T], f32, tag="x")
                nc.sync.dma_start(xq[:], xT[ts(ib, P), :])
                xb = xt_pool.tile([P, T], bf16, tag=f"xt_{ib}", name=f"xt_{ib}")
                nc.scalar.copy(xb[:], xq[:])
                xt_tiles.append(xb)

            # ---- per o-chunk: quantize w^T strips, then matmul sweep ----
            for c in range(n_oc):
                qt_tiles = []
                for ib in range(n_ib):
                    wst = win_pool.tile([P, OC], f32, tag="w")
                    nc.sync.dma_start(wst[:], wT[ts(ib, P), ds(c * OC, OC)])
                    t2 = tq_pool.tile([P, OC], bf16, tag="t2")
                    nc.vector.tensor_scalar(
                        out=t2[:],
                        in0=wst[:],
                        scalar1=thr[:],
                        scalar2=2.0,
                        op0=A.is_gt,
                        op1=A.mult,
                    )
                    s2 = sq_pool.tile([P, OC], bf16, tag="s2")
                    nc.scalar.activation(
                        s2[:],
                        wst[:],
                        mybir.ActivationFunctionType.Sign,
                        bias=thr[:],
                    )
                    q2 = qt_pool.tile(
                        [P, OC], bf16, tag=f"qt_{ib}", name=f"qt_{c}_{ib}"
                    )
                    # q2 = (t2 - 1) + s2  in {-2, 0, 2}  (= 2q)
                    nc.vector.scalar_tensor_tensor(
                        out=q2[:],
                        in0=t2[:],
                        scalar=-1.0,
                        in1=s2[:],
                        op0=A.add,
                        op1=A.add,
                    )
                    qt_tiles.append(q2)
                for tb in range(n_tb):
                    ps = ps_acc.tile([P, OC], f32, tag="acc")
                    for ib in range(n_ib):
                        nc.tensor.matmul(
                            ps[:],
                            lhsT=xt_tiles[ib][:, ts(tb, P)],
                            rhs=qt_tiles[ib][:],
                            start=(ib == 0),
                            stop=(ib == n_ib - 1),
                        )
                    osb = out_pool.tile([P, OC], f32, tag="o")
                    # psum holds x @ (2q)^T; scale by thr = s_w/2
                    nc.scalar.activation(
                        osb[:],
                        ps[:],
                        mybir.ActivationFunctionType.Copy,
                        scale=thr[:],
                    )
                    nc.sync.dma_start(out_sh[ts(tb, P), ds(c * OC, OC)], osb[:])

    nc.compile()
    return nc


_CACHE = {}


def _get_nc(key):
    if key not in _CACHE:
        _CACHE[key] = build_nc(*key)
    return _CACHE[key]


def make_in_maps(x2d, weight, n_cores=N_CORES, tg=TG, fg=FG):
    """Host-side sharding: per-core pre-transposed inputs."""
    t_tot, i_full = x2d.shape
    o_full = weight.shape[0]
    t_sh = t_tot // tg
    o_sh = o_full // fg
    i_slab = i_full // tg
    wT_halves = {}
    for b in range(fg):
        wT_halves[b] = np.ascontiguousarray(weight[b * o_sh : (b + 1) * o_sh].T)
    in_maps = []
    for cid in range(n_cores):
        g, b = cid // fg, cid % fg
        # rotate i-rows of wT so rows [0, i_slab) are this core's slab;
        # the matmul contraction is a sum over i, invariant to the
        # rotation as long as xT rows are rotated identically.
        roll = -g * i_slab
        in_maps.append(
            {
                "xT": np.ascontiguousarray(
                    np.roll(x2d[g * t_sh : (g + 1) * t_sh].T, roll, axis=0)
                ),
                "wT": np.roll(wT_halves[b], roll, axis=0),
            }
        )
    return in_maps


def run(x2d, weight, n_cores=N_CORES, tg=TG, fg=FG):
    """Run the sharded device computation: returns x @ q^T * s_w, [Ttot, O_full]."""
    from concourse.bass_utils import run_bass_kernel_spmd

    t_tot, i_full = x2d.shape
    o_full = weight.shape[0]
    t_sh = t_tot // tg
    o_sh = o_full // fg
    key = (t_sh, o_sh, i_full, n_cores, tg, o_full * i_full)
    nc = _get_nc(key)

    in_maps = make_in_maps(x2d, weight, n_cores, tg, fg)
    res = run_bass_kernel_spmd(nc, in_maps, core_ids=list(range(n_cores)))
    out = np.empty((t_tot, o_full), np.float32)
    for cid in range(n_cores):
        g, b = cid // fg, cid % fg
        out[g * t_sh : (g + 1) * t_sh, b * o_sh : (b + 1) * o_sh] = res.results[
            cid
        ]["out_sh"]
    return out


def kernel(x, weight, bias):
    x = np.asarray(x, np.float32)
    weight = np.asarray(weight, np.float32)
    bias = np.asarray(bias, np.float32)
    t_tot = x.shape[0] * x.shape[1]
    out = run(x.reshape(t_tot, x.shape[2]), weight)
    # bias term: out += bias * s_x (exact reference semantics; zero for
    # this problem's bias). The matmul term is s_x-invariant.
    if np.any(bias):
        s_x = np.float32(max(np.mean(np.abs(x)), EPS))
        out = out + (bias * s_x)[None, :]
    return out.reshape(x.shape[0], x.shape[1], weight.shape[0])
